# revision 1
# baseline (speedup 1.0000x reference)
"""MeshPool (greedy heap-order edge collapse) for Trainium2, 8 NeuronCores.

Structure:
  * The edge pop order is argsort(||f[v0]||^2 + ||f[v1]||^2). The reference
    runs under jax on CPU, and float ties/near-ties in the priorities make the
    merge cascade sensitive to the exact f32 bits, so the priorities are
    computed with jax pinned to the CPU backend (bit-identical to the
    reference's arithmetic), then stably argsorted.
  * The collapse scan itself is integer-only (neighbor counts, alive flags)
    and inherently sequential -> replayed exactly in numpy on the host,
    recording the executed merges. Merge weights are powers of two, so the
    sparse replay is exact.
  * The heavy, parallel part - pooled = (A @ features) * alive with dense
    A [V, V] - runs on the 8 NeuronCores: A^T is built on the host (dead
    rows of A dropped, i.e. pre-masked), sharded by output rows, and each
    core computes P_shard^T = F^T @ A_T_shard as a PSUM-accumulated matmul.
"""

import numpy as np

V, D, SH = 4096, 128, 512  # vertices, feature dim, rows per core
KT = V // 128              # contraction tiles
N_CORES = 8

_nc_cache = {}


def _edge_order(features, edges):
    """Pop order of the reference's heap, bit-matching jax-on-CPU f32 math."""
    import jax
    import jax.numpy as jnp

    cpu = jax.devices("cpu")[0]
    with jax.default_device(cpu):
        f = jnp.asarray(np.ascontiguousarray(features, dtype=np.float32))
        e = jnp.asarray(np.ascontiguousarray(edges)).astype(jnp.int32)
        pri = jnp.sum(f * f, axis=1)
        epri = pri[e[0]] + pri[e[1]]
    epri = np.asarray(epri)
    return np.argsort(epri, kind="stable")  # stable == jnp.argsort


def _collapse(edges, neighbor, order):
    """Exact numpy replay of the reference scan. Returns (merges, alive, cnt)."""
    target = V // 2
    n = np.ascontiguousarray(neighbor, dtype=np.int32).copy()
    alive = np.ones(V, bool)
    cnt = V
    merges = []
    e0 = edges[0].astype(np.int64)
    e1 = edges[1].astype(np.int64)
    for e in order:
        v0 = int(e0[e])
        v1 = int(e1[e])
        if cnt > target and alive[v0] and alive[v1] and n[v0, v1] == 2:
            r = n[v0] + n[v1]
            r[v0] = 0
            r[v1] = 0
            n[v0, :] = r
            n[:, v0] = r
            n[v1, :] = 0
            n[:, v1] = 0
            alive[v1] = False
            cnt -= 1
            merges.append((v0, v1))
    return merges, alive, cnt


def _merge_weights(merges):
    """Final A rows as {orig_vertex: weight}; weights are exact powers of 2."""
    W = {}
    for v0, v1 in merges:
        w0 = W.get(v0) or {v0: 1.0}
        w1 = W.get(v1) or {v1: 1.0}
        m = {u: 0.5 * w for u, w in w0.items()}
        for u, w in w1.items():
            m[u] = m.get(u, 0.0) + 0.5 * w
        W[v0] = m
    return W


def _build_a_t(merges, alive):
    """Dense A^T [V, V] f32 with dead output rows (columns here) zeroed."""
    a_t = np.zeros((V, V), dtype=np.float32)
    idx = np.flatnonzero(alive)
    a_t[idx, idx] = 1.0
    for v, wd in _merge_weights(merges).items():
        if alive[v]:
            a_t[:, v] = 0.0
            for u, w in wd.items():
                a_t[u, v] = w
    return a_t


def _build_nc():
    import concourse.bacc as bacc
    import concourse.mybir as mybir
    import concourse.tile as tile

    nc = bacc.Bacc("TRN2", target_bir_lowering=False, debug=False,
                   num_devices=N_CORES)
    a_t = nc.dram_tensor("a_t", [V, SH], mybir.dt.float32,
                         kind="ExternalInput").ap()
    f = nc.dram_tensor("f", [V, D], mybir.dt.float32,
                       kind="ExternalInput").ap()
    p_t = nc.dram_tensor("p_t", [D, SH], mybir.dt.float32,
                         kind="ExternalOutput").ap()

    with tile.TileContext(nc) as tc:
        with tc.tile_pool(name="fpool", bufs=1) as fpool, \
             tc.tile_pool(name="apool", bufs=KT) as apool, \
             tc.tile_pool(name="psum", bufs=2, space="PSUM") as psum_pool, \
             tc.tile_pool(name="opool", bufs=1) as opool:
            # F resident: fsb[p, k, d] = F[k*128 + p, d]
            fsb = fpool.tile([128, KT, D], mybir.dt.float32)
            nc.sync.dma_start(out=fsb, in_=f.rearrange("(k p) d -> p k d", p=128))
            # A PE instruction carries a single sem wait; this warm-up matmul
            # reads only fsb so the PE observes its DMA before the real chain
            # (whose matmuls then wait only on their own A-tile DMA).
            warm = psum_pool.tile([D, D], mybir.dt.float32)
            nc.tensor.matmul(out=warm, lhsT=fsb[:, 0, :], rhs=fsb[:, 0, :],
                             start=True, stop=True)
            ps = psum_pool.tile([D, SH], mybir.dt.float32)
            for k in range(KT):
                at = apool.tile([128, SH], mybir.dt.float32)
                nc.sync.dma_start(out=at, in_=a_t[k * 128:(k + 1) * 128, :])
                nc.tensor.matmul(out=ps, lhsT=fsb[:, k, :], rhs=at,
                                 start=(k == 0), stop=(k == KT - 1))
            ot = opool.tile([D, SH], mybir.dt.float32)
            nc.vector.tensor_copy(out=ot, in_=ps)
            nc.gpsimd.dma_start(out=p_t, in_=ot)
    nc.compile()
    return nc


def _run_device(a_t_full, features, trace=False, tmpdir=None):
    from concourse.bass_utils import run_bass_kernel_spmd

    if "nc" not in _nc_cache:
        _nc_cache["nc"] = _build_nc()
    nc = _nc_cache["nc"]
    f32 = np.ascontiguousarray(features, dtype=np.float32)
    in_maps = [
        {"a_t": np.ascontiguousarray(a_t_full[:, c * SH:(c + 1) * SH]),
         "f": f32}
        for c in range(N_CORES)
    ]
    res = run_bass_kernel_spmd(nc, in_maps, core_ids=list(range(N_CORES)),
                               trace=trace, tmpdir=tmpdir)
    pooled = np.empty((V, D), dtype=np.float32)
    for c in range(N_CORES):
        pooled[c * SH:(c + 1) * SH, :] = res.results[c]["p_t"].T
    return pooled, res


def kernel(features, edges, neighbor, _trace=False, _tmpdir=None):
    features = np.asarray(features)
    edges = np.asarray(edges)
    neighbor = np.asarray(neighbor)
    assert neighbor.shape == (V, V) and features.shape == (V, D)

    order = _edge_order(features, edges)
    merges, alive, cnt = _collapse(edges, neighbor, order)
    a_t = _build_a_t(merges, alive)
    pooled, res = _run_device(a_t, features, trace=_trace, tmpdir=_tmpdir)
    out = (pooled, alive, np.int32(cnt))
    if _trace:
        return out, res
    return out


# revision 2
# speedup vs baseline: 1.1900x; 1.1900x over previous
"""MeshPool (greedy heap-order edge collapse) for Trainium2, 8 NeuronCores.

Structure:
  * The edge pop order is argsort(||f[v0]||^2 + ||f[v1]||^2). The reference
    runs under jax on CPU, and float ties/near-ties in the priorities make the
    merge cascade sensitive to the exact f32 bits, so the priorities are
    computed with jax pinned to the CPU backend (bit-identical to the
    reference's arithmetic), then stably argsorted.
  * The collapse scan itself is integer-only (neighbor counts, alive flags)
    and inherently sequential -> replayed exactly in numpy on the host,
    recording the executed merges. Merge weights are powers of two, so the
    sparse replay is exact.
  * The heavy, parallel part - pooled = (A @ features) * alive with dense
    A [V, V] - runs on the 8 NeuronCores: A^T is built on the host (dead
    rows of A dropped, i.e. pre-masked), sharded by output rows, and each
    core computes P_shard^T = F^T @ A_T_shard as a PSUM-accumulated matmul.
"""

import numpy as np

V, D, SH = 4096, 128, 512  # vertices, feature dim, rows per core
KT = V // 128              # contraction tiles
N_CORES = 8

_nc_cache = {}


def _edge_order(features, edges):
    """Pop order of the reference's heap, bit-matching jax-on-CPU f32 math."""
    import jax
    import jax.numpy as jnp

    cpu = jax.devices("cpu")[0]
    with jax.default_device(cpu):
        f = jnp.asarray(np.ascontiguousarray(features, dtype=np.float32))
        e = jnp.asarray(np.ascontiguousarray(edges)).astype(jnp.int32)
        pri = jnp.sum(f * f, axis=1)
        epri = pri[e[0]] + pri[e[1]]
    epri = np.asarray(epri)
    return np.argsort(epri, kind="stable")  # stable == jnp.argsort


def _collapse(edges, neighbor, order):
    """Exact numpy replay of the reference scan. Returns (merges, alive, cnt)."""
    target = V // 2
    n = np.ascontiguousarray(neighbor, dtype=np.int32).copy()
    alive = np.ones(V, bool)
    cnt = V
    merges = []
    e0 = edges[0].astype(np.int64)
    e1 = edges[1].astype(np.int64)
    for e in order:
        v0 = int(e0[e])
        v1 = int(e1[e])
        if cnt > target and alive[v0] and alive[v1] and n[v0, v1] == 2:
            r = n[v0] + n[v1]
            r[v0] = 0
            r[v1] = 0
            n[v0, :] = r
            n[:, v0] = r
            n[v1, :] = 0
            n[:, v1] = 0
            alive[v1] = False
            cnt -= 1
            merges.append((v0, v1))
    return merges, alive, cnt


def _merge_weights(merges):
    """Final A rows as {orig_vertex: weight}; weights are exact powers of 2."""
    W = {}
    for v0, v1 in merges:
        w0 = W.get(v0) or {v0: 1.0}
        w1 = W.get(v1) or {v1: 1.0}
        m = {u: 0.5 * w for u, w in w0.items()}
        for u, w in w1.items():
            m[u] = m.get(u, 0.0) + 0.5 * w
        W[v0] = m
    return W


def _build_a_t(merges, alive):
    """Dense A^T [V, V] f32 with dead output rows (columns here) zeroed."""
    a_t = np.zeros((V, V), dtype=np.float32)
    idx = np.flatnonzero(alive)
    a_t[idx, idx] = 1.0
    for v, wd in _merge_weights(merges).items():
        if alive[v]:
            a_t[:, v] = 0.0
            for u, w in wd.items():
                a_t[u, v] = w
    return a_t


def _build_nc():
    import concourse.bacc as bacc
    import concourse.mybir as mybir
    import concourse.tile as tile

    nc = bacc.Bacc("TRN2", target_bir_lowering=False, debug=False,
                   num_devices=N_CORES)
    # A^T shard in bf16: merge weights are powers of two, so the cast is
    # exact. F is shipped as a bf16 (hi, lo) pair stacked along rows:
    # F = hi + lo to ~2^-17 relative, and the PSUM accumulation is fp32.
    a_t = nc.dram_tensor("a_t", [V, SH], mybir.dt.bfloat16,
                         kind="ExternalInput").ap()
    f2 = nc.dram_tensor("f2", [2 * V, D], mybir.dt.bfloat16,
                        kind="ExternalInput").ap()
    p_t = nc.dram_tensor("p_t", [D, SH], mybir.dt.float32,
                         kind="ExternalOutput").ap()

    with tile.TileContext(nc) as tc:
        with tc.tile_pool(name="fpool", bufs=1) as fpool, \
             tc.tile_pool(name="apool", bufs=KT) as apool, \
             tc.tile_pool(name="psum", bufs=2, space="PSUM") as psum_pool, \
             tc.tile_pool(name="opool", bufs=1) as opool:
            # F hi/lo resident: fsb[p, j, d] = F2[j*128 + p, d], j < 2*KT
            fsb = fpool.tile([128, 2 * KT, D], mybir.dt.bfloat16)
            nc.sync.dma_start(out=fsb, in_=f2.rearrange("(j p) d -> p j d", p=128))
            # A PE instruction carries a single sem wait; this warm-up matmul
            # reads only fsb so the PE observes its DMA before the real chain
            # (whose matmuls then wait only on their own A-tile DMA).
            warm = psum_pool.tile([D, D], mybir.dt.float32)
            nc.tensor.matmul(out=warm, lhsT=fsb[:, 0, :], rhs=fsb[:, 0, :D],
                             start=True, stop=True)
            ps = psum_pool.tile([D, SH], mybir.dt.float32)
            for k in range(KT):
                at = apool.tile([128, SH], mybir.dt.bfloat16)
                nc.sync.dma_start(out=at, in_=a_t[k * 128:(k + 1) * 128, :])
                nc.tensor.matmul(out=ps, lhsT=fsb[:, k, :], rhs=at,
                                 start=(k == 0), stop=False)
                nc.tensor.matmul(out=ps, lhsT=fsb[:, KT + k, :], rhs=at,
                                 start=False, stop=(k == KT - 1))
            ot = opool.tile([D, SH], mybir.dt.float32)
            nc.vector.tensor_copy(out=ot, in_=ps)
            nc.gpsimd.dma_start(out=p_t, in_=ot)
    nc.compile()
    return nc


def _run_device(a_t_full, features, trace=False, tmpdir=None):
    import ml_dtypes
    from concourse.bass_utils import run_bass_kernel_spmd

    if "nc" not in _nc_cache:
        _nc_cache["nc"] = _build_nc()
    nc = _nc_cache["nc"]
    f32 = np.ascontiguousarray(features, dtype=np.float32)
    f_hi = f32.astype(ml_dtypes.bfloat16)
    f_lo = (f32 - f_hi.astype(np.float32)).astype(ml_dtypes.bfloat16)
    f2 = np.ascontiguousarray(np.concatenate([f_hi, f_lo], axis=0))
    a_t_bf = a_t_full.astype(ml_dtypes.bfloat16)
    in_maps = [
        {"a_t": np.ascontiguousarray(a_t_bf[:, c * SH:(c + 1) * SH]),
         "f2": f2}
        for c in range(N_CORES)
    ]
    res = run_bass_kernel_spmd(nc, in_maps, core_ids=list(range(N_CORES)),
                               trace=trace, tmpdir=tmpdir)
    pooled = np.empty((V, D), dtype=np.float32)
    for c in range(N_CORES):
        pooled[c * SH:(c + 1) * SH, :] = res.results[c]["p_t"].T
    return pooled, res


def kernel(features, edges, neighbor, _trace=False, _tmpdir=None):
    features = np.asarray(features)
    edges = np.asarray(edges)
    neighbor = np.asarray(neighbor)
    assert neighbor.shape == (V, V) and features.shape == (V, D)

    order = _edge_order(features, edges)
    merges, alive, cnt = _collapse(edges, neighbor, order)
    a_t = _build_a_t(merges, alive)
    pooled, res = _run_device(a_t, features, trace=_trace, tmpdir=_tmpdir)
    out = (pooled, alive, np.int32(cnt))
    if _trace:
        return out, res
    return out


# revision 4
# speedup vs baseline: 1.2799x; 1.0755x over previous
"""MeshPool (greedy heap-order edge collapse) for Trainium2, 8 NeuronCores.

Structure:
  * The edge pop order is argsort(||f[v0]||^2 + ||f[v1]||^2). The reference
    runs under jax on CPU, and float ties/near-ties in the priorities make the
    merge cascade sensitive to the exact f32 bits, so the priorities are
    computed with jax pinned to the CPU backend (bit-identical to the
    reference's arithmetic), then stably argsorted.
  * The collapse scan itself is integer-only (neighbor counts, alive flags)
    and inherently sequential -> replayed exactly in numpy on the host,
    recording the executed merges. Merge weights are powers of two, so the
    sparse replay is exact.
  * The heavy, parallel part - pooled = (A @ features) * alive with dense
    A [V, V] - runs on the 8 NeuronCores: A^T is built on the host (dead
    rows of A dropped, i.e. pre-masked), sharded by output rows, and each
    core computes P_shard^T = F^T @ A_T_shard as a PSUM-accumulated matmul.
"""

import numpy as np

V, D, SH = 4096, 128, 512  # vertices, feature dim, rows per core
KT = V // 128              # contraction tiles
N_CORES = 8

_nc_cache = {}


def _edge_order(features, edges):
    """Pop order of the reference's heap, bit-matching jax-on-CPU f32 math."""
    import jax
    import jax.numpy as jnp

    cpu = jax.devices("cpu")[0]
    with jax.default_device(cpu):
        f = jnp.asarray(np.ascontiguousarray(features, dtype=np.float32))
        e = jnp.asarray(np.ascontiguousarray(edges)).astype(jnp.int32)
        pri = jnp.sum(f * f, axis=1)
        epri = pri[e[0]] + pri[e[1]]
    epri = np.asarray(epri)
    return np.argsort(epri, kind="stable")  # stable == jnp.argsort


def _collapse(edges, neighbor, order):
    """Exact numpy replay of the reference scan. Returns (merges, alive, cnt)."""
    target = V // 2
    n = np.ascontiguousarray(neighbor, dtype=np.int32).copy()
    alive = np.ones(V, bool)
    cnt = V
    merges = []
    e0 = edges[0].astype(np.int64)
    e1 = edges[1].astype(np.int64)
    for e in order:
        v0 = int(e0[e])
        v1 = int(e1[e])
        if cnt > target and alive[v0] and alive[v1] and n[v0, v1] == 2:
            r = n[v0] + n[v1]
            r[v0] = 0
            r[v1] = 0
            n[v0, :] = r
            n[:, v0] = r
            n[v1, :] = 0
            n[:, v1] = 0
            alive[v1] = False
            cnt -= 1
            merges.append((v0, v1))
    return merges, alive, cnt


def _merge_weights(merges):
    """Final A rows as {orig_vertex: weight}; weights are exact powers of 2."""
    W = {}
    for v0, v1 in merges:
        w0 = W.get(v0) or {v0: 1.0}
        w1 = W.get(v1) or {v1: 1.0}
        m = {u: 0.5 * w for u, w in w0.items()}
        for u, w in w1.items():
            m[u] = m.get(u, 0.0) + 0.5 * w
        W[v0] = m
    return W


def _build_a_t(merges, alive):
    """Dense A^T [V, V] f32 with dead output rows (columns here) zeroed."""
    a_t = np.zeros((V, V), dtype=np.float32)
    idx = np.flatnonzero(alive)
    a_t[idx, idx] = 1.0
    for v, wd in _merge_weights(merges).items():
        if alive[v]:
            a_t[:, v] = 0.0
            for u, w in wd.items():
                a_t[u, v] = w
    return a_t


def _build_nc():
    import concourse.bacc as bacc
    import concourse.mybir as mybir
    import concourse.tile as tile

    nc = bacc.Bacc("TRN2", target_bir_lowering=False, debug=False,
                   num_devices=N_CORES)
    # Both operands are pre-tiled on the host into partition-major layouts so
    # every DMA is a [128, wide] transfer with large contiguous runs per
    # partition (descriptor-efficient):
    #   f2t[p, j*D + d]  = Fhi/lo[j*128 + p, d]   (j < 2*KT; hi then lo)
    #   a_tt[p, k*SH + j] = A_T[k*128 + p, j]
    # A^T in bf16 is exact (merge weights are powers of two); F is a bf16
    # (hi, lo) pair with fp32 PSUM accumulation.
    f2t = nc.dram_tensor("f2t", [128, 2 * KT * D], mybir.dt.bfloat16,
                         kind="ExternalInput").ap()
    a_tt = nc.dram_tensor("a_tt", [128, KT * SH], mybir.dt.bfloat16,
                          kind="ExternalInput").ap()
    p_t = nc.dram_tensor("p_t", [D, SH], mybir.dt.float32,
                         kind="ExternalOutput").ap()
    GRP = 8  # k-tiles per A-group DMA

    with tile.TileContext(nc) as tc:
        with tc.tile_pool(name="fpool", bufs=1) as fpool, \
             tc.tile_pool(name="apool", bufs=KT // GRP) as apool, \
             tc.tile_pool(name="psum", bufs=2, space="PSUM") as psum_pool, \
             tc.tile_pool(name="opool", bufs=1) as opool:
            fsb = fpool.tile([128, 2 * KT * D], mybir.dt.bfloat16)
            nc.sync.dma_start(out=fsb, in_=f2t)
            # A PE instruction carries a single sem wait; this warm-up matmul
            # reads only fsb so the PE observes its DMA before the real chain
            # (whose matmuls then wait only on their own A-group DMA).
            warm = psum_pool.tile([D, D], mybir.dt.float32)
            nc.tensor.matmul(out=warm, lhsT=fsb[:, 0:D], rhs=fsb[:, 0:D],
                             start=True, stop=True)
            ps = psum_pool.tile([D, SH], mybir.dt.float32)
            for g in range(KT // GRP):
                ag = apool.tile([128, GRP * SH], mybir.dt.bfloat16)
                nc.sync.dma_start(
                    out=ag, in_=a_tt[:, g * GRP * SH:(g + 1) * GRP * SH])
                for kk in range(GRP):
                    k = g * GRP + kk
                    at = ag[:, kk * SH:(kk + 1) * SH]
                    nc.tensor.matmul(out=ps, lhsT=fsb[:, k * D:(k + 1) * D],
                                     rhs=at, start=(k == 0), stop=False)
                    nc.tensor.matmul(
                        out=ps, lhsT=fsb[:, (KT + k) * D:(KT + k + 1) * D],
                        rhs=at, start=False, stop=(k == KT - 1))
            ot = opool.tile([D, SH], mybir.dt.float32)
            nc.vector.tensor_copy(out=ot, in_=ps)
            nc.gpsimd.dma_start(out=p_t, in_=ot)
    nc.compile()
    return nc


def _run_device(a_t_full, features, trace=False, tmpdir=None):
    import ml_dtypes
    from concourse.bass_utils import run_bass_kernel_spmd

    if "nc" not in _nc_cache:
        _nc_cache["nc"] = _build_nc()
    nc = _nc_cache["nc"]
    f32 = np.ascontiguousarray(features, dtype=np.float32)
    f_hi = f32.astype(ml_dtypes.bfloat16)
    f_lo = (f32 - f_hi.astype(np.float32)).astype(ml_dtypes.bfloat16)
    # [2V, D] -> partition-major [128, 2*KT*D]
    f2 = np.concatenate([f_hi, f_lo], axis=0)
    f2t = np.ascontiguousarray(
        f2.reshape(2 * KT, 128, D).transpose(1, 0, 2).reshape(128, 2 * KT * D))
    a_t_bf = a_t_full.astype(ml_dtypes.bfloat16)
    in_maps = []
    for c in range(N_CORES):
        sh = a_t_bf[:, c * SH:(c + 1) * SH]  # [V, SH]
        a_tt = np.ascontiguousarray(
            sh.reshape(KT, 128, SH).transpose(1, 0, 2).reshape(128, KT * SH))
        in_maps.append({"a_tt": a_tt, "f2t": f2t})
    res = run_bass_kernel_spmd(nc, in_maps, core_ids=list(range(N_CORES)),
                               trace=trace, tmpdir=tmpdir)
    pooled = np.empty((V, D), dtype=np.float32)
    for c in range(N_CORES):
        pooled[c * SH:(c + 1) * SH, :] = res.results[c]["p_t"].T
    return pooled, res


def kernel(features, edges, neighbor, _trace=False, _tmpdir=None):
    features = np.asarray(features)
    edges = np.asarray(edges)
    neighbor = np.asarray(neighbor)
    assert neighbor.shape == (V, V) and features.shape == (V, D)

    order = _edge_order(features, edges)
    merges, alive, cnt = _collapse(edges, neighbor, order)
    a_t = _build_a_t(merges, alive)
    pooled, res = _run_device(a_t, features, trace=_trace, tmpdir=_tmpdir)
    out = (pooled, alive, np.int32(cnt))
    if _trace:
        return out, res
    return out
